# revision 1
# baseline (speedup 1.0000x reference)
"""Trainium2 Bass kernel for the CustomCRFLoss problem.

Strategy (pure data parallel, one sample per NeuronCore, 8 cores):

The reference computes, per sample:
    unary  = softplus(d) - label*d            (d = l1 - l0, 2 classes)
    val[i,j,w'] = exp(-di/2) + exp(-dj/2)     (128^3 pairwise Gaussian kernels)
    5 x mean-field:  Q <- Q - (P@Q)           (P@Q)[i,j] = sum_w val[i,j,w]*colsum(Q)[w]
    out = mean(Q)

Key reduction: the output only needs column sums.  With
    M[j,w] = sum_i val[i,j,w]   (a single 128x128 matrix per sample)
the 5 iterations collapse to 5 matvecs:
    q_{t+1} = q_t - M q_t,  answer = (sum(unary) - sum_t 1^T M q_t) / (n*h*w)

M = M1 + M2:
  * spatial part M1[j,w] = sum_i exp(-||x_ij - x_iw||^2/2) is computed with a
    degree-5 Taylor feature map phi_m(x) = x^alpha/sqrt(alpha!) * exp(-r/2)
    (x centered, so |<x,x'>| <= 0.75 and the truncation error is ~2e-4):
    M1 = sum_m T_m^T T_m  -- 56 bf16 matmuls accumulated in PSUM, zero exps.
  * bilateral part M2[j,w] = sum_i exp(-||x_ij - x_wj||^2/2) via 128 small
    K=5 bf16 Gram matmuls (augmented with -r/2 rows so PSUM holds the full
    exponent), batched ACT exp, and DVE row-sum reduction (the per-column
    kernel matrix is symmetric, so row sums equal the needed column sums).

Emulated accuracy vs the JAX reference: rel err ~2.5e-5.
"""

import math

import numpy as np

import concourse.bass as bass
import concourse.tile as tile
from concourse import mybir
from concourse.bass_utils import run_bass_kernel_spmd
from concourse.tile import add_dep_helper

H = W = 128
PIX = H * W
NB = 8  # batch / cores
DEG = 4

F32 = mybir.dt.float32
BF16 = mybir.dt.bfloat16
AF = mybir.ActivationFunctionType
ALU = mybir.AluOpType
AX = mybir.AxisListType

# bilateral grouping: GRP j-columns per PSUM group tile
GRP = 8
NGRP = W // GRP


def _monomial_ops(deg):
    """Canonically ordered monomials of degree<=deg in 3 vars + build schedule.

    Order within degree k: [x1^k] + x2*(c0-prefix of L(k-1)) + x3*L(k-1).
    This makes every op's parents AND children contiguous ranges, and the
    x3-children's scale (1/sqrt(c+1)) constant on runs of equal parent c.
    Returns (total_count, ops) with ops = (child_off, parent_off, width,
    channel, scale); each feature tile ends up as x^alpha/sqrt(alpha!)*E0.
    """
    L = [[(0, 0, 0)]]
    for k in range(1, deg + 1):
        prev = L[-1]
        cur = [(prev[0][0] + 1, prev[0][1], prev[0][2])]
        cur += [(a, b + 1, 0) for (a, b, c) in prev if c == 0]
        cur += [(a, b, c + 1) for (a, b, c) in prev]
        L.append(cur)
    offs = [0]
    for lst in L:
        offs.append(offs[-1] + len(lst))
    ops = []
    for k in range(1, deg + 1):
        po, co = offs[k - 1], offs[k]
        prev = L[k - 1]
        ops.append((co, po, 1, 0, k))
        for t in range(k):
            ops.append((co + 1 + t, po + t, 1, 1, t + 1))
        base = co + 1 + k
        i = 0
        while i < len(prev):
            cval = prev[i][2]
            jx = i
            while jx < len(prev) and prev[jx][2] == cval:
                jx += 1
            ops.append((base + i, po + i, jx - i, 2, cval + 1))
            i = jx
    return offs[-1], ops


NMON, MONOPS = _monomial_ops(DEG)


def _bcast(ap, wid):
    """[P,128] AP -> [P,wid,128] with a step-0 middle dim."""
    return bass.AP(
        tensor=ap.tensor,
        offset=ap.offset,
        ap=[list(ap.ap[0]), [0, wid], list(ap.ap[1])],
    )


def build_kernel():
    nc = bass.Bass()
    lg_d = nc.dram_tensor("logits", (2, H, W), F32, kind="ExternalInput")
    lb_d = nc.dram_tensor("labels", (H, W), F32, kind="ExternalInput")
    im_d = nc.dram_tensor("imb", (3, H, W), BF16, kind="ExternalInput")
    imT_d = nc.dram_tensor("imtb", (3, W, H), BF16, kind="ExternalInput")
    out_d = nc.dram_tensor("out", (1, H), F32, kind="ExternalOutput")

    with tile.TileContext(nc) as tc:
        with (
            tc.tile_pool(name="sb", bufs=1) as sb,
            tc.tile_pool(name="ex", bufs=3) as expp,
            tc.tile_pool(name="qp", bufs=3) as qpool,
            tc.tile_pool(name="pg", bufs=2, space="PSUM") as pg,
            tc.tile_pool(name="pm", bufs=1, space="PSUM") as pm,
            tc.tile_pool(name="pm2", bufs=1, space="PSUM") as pm2,
            tc.tile_pool(name="ps", bufs=1, space="PSUM") as ps,
        ):
            # -------- inputs: bf16 pre-centered images, both layouts --------
            # J row order: JL = [ones, nrT, x1T, x2T, x3T]
            #              JR = [nrT, ones, x1T, x2T, x3T]
            # pairs: 1*nrT (free side) + nrT*1 (partition side) + x*x = exponent
            JL = sb.tile([5, PIX], BF16)
            JR = sb.tile([5, PIX], BF16)
            ttile = sb.tile([W, 3, H], BF16)
            nc.sync.dma_start(out=ttile, in_=imT_d[:].rearrange("c j i -> j c i"))
            imtile = sb.tile([H, 3, W], BF16)
            nc.scalar.dma_start(out=imtile, in_=im_d[:].rearrange("c i j -> i c j"))
            nc.sync.dma_start(
                out=JL[2:5, :], in_=imT_d[:].rearrange("c j i -> c (j i)")
            )
            nc.scalar.dma_start(
                out=JR[2:5, :], in_=imT_d[:].rearrange("c j i -> c (j i)")
            )
            xb = [imtile[:, c, :] for c in range(3)]
            tbs = [ttile[:, c, :] for c in range(3)]
            # ---------------- constants ----------------
            ones_col = sb.tile([H, 1], F32)
            nc.vector.memset(ones_col, 1.0)
            ones_b = sb.tile([H, W], BF16)
            nc.vector.memset(ones_b, 1.0)
            nc.gpsimd.dma_start(out=JL[0:1, :], in_=ones_b)
            nc.gpsimd.dma_start(out=JR[1:2, :], in_=ones_b)
            ones_cb = sb.tile([H, 1], BF16)
            nc.vector.memset(ones_cb, 1.0)

            # ---------------- nrT (gates bilateral), then nr ----------------
            def _sumsq(srcs, tag):
                s1 = sb.tile([H, W], F32, tag=f"{tag}1")
                nc.vector.tensor_mul(out=s1, in0=srcs[0], in1=srcs[0])
                s2 = sb.tile([H, W], F32, tag=f"{tag}2")
                nc.vector.tensor_mul(out=s2, in0=srcs[1], in1=srcs[1])
                s12 = sb.tile([H, W], F32, tag=f"{tag}3")
                nc.vector.tensor_add(out=s12, in0=s1, in1=s2)
                s3 = sb.tile([H, W], F32, tag=f"{tag}4")
                nc.vector.tensor_mul(out=s3, in0=srcs[2], in1=srcs[2])
                o = sb.tile([H, W], F32, tag=f"{tag}5")
                nc.vector.tensor_add(out=o, in0=s12, in1=s3)
                return o

            rT = _sumsq(tbs, "rT")
            nrT_b = sb.tile([W, H], BF16)
            nc.vector.tensor_scalar_mul(out=nrT_b, in0=rT, scalar1=-0.5)
            nc.sync.dma_start(out=JL[1:2, :], in_=nrT_b)
            nc.gpsimd.dma_start(out=JR[0:1, :], in_=nrT_b)
            lg = sb.tile([H, 2, W], F32)
            nc.sync.dma_start(out=lg, in_=lg_d[:].rearrange("c i j -> i c j"))
            lb = sb.tile([H, W], F32)
            nc.sync.dma_start(out=lb, in_=lb_d[:])
            rr = _sumsq(xb, "rr")
            nr = sb.tile([H, W], F32)
            nc.vector.tensor_scalar_mul(out=nr, in0=rr, scalar1=-0.5)

            # ---------------- bilateral: G -> exp -> rowsum ----------------
            # mt2[w', j] = sum_i E_j[i, w']  (E_j symmetric -> row sums)
            mt2 = pm2.tile([H, W], F32)
            gmm_first = {}
            for g in range(NGRP):
                gp = pg.tile([H, GRP, W], F32, tag="g")
                for t in range(GRP):
                    j = g * GRP + t
                    _mm = nc.tensor.matmul(
                        gp[:, t, :],
                        lhsT=JL[:, j * W : (j + 1) * W],
                        rhs=JR[:, j * W : (j + 1) * W],
                        start=True,
                        stop=True,
                    )
                    if t == 0:
                        gmm_first[g] = _mm.ins
                ech = expp.tile([H, GRP, W], BF16, tag="ech")
                nc.scalar.activation(out=ech, in_=gp, func=AF.Exp)
                for t in range(GRP):
                    j = g * GRP + t
                    nc.tensor.matmul(
                        mt2[:, j : j + 1],
                        lhsT=ech[:, t, :],
                        rhs=ones_cb,
                        start=True,
                        stop=True,
                    )

            # ---------------- unary ----------------
            dd = sb.tile([H, W], F32)
            nc.vector.tensor_sub(out=dd, in0=lg[:, 1, :], in1=lg[:, 0, :])
            ed = sb.tile([H, W], F32)
            nc.scalar.activation(out=ed, in_=dd, func=AF.Exp)
            sp = sb.tile([H, W], F32)
            nc.scalar.activation(out=sp, in_=ed, func=AF.Ln, bias=1.0)
            tl = sb.tile([H, W], F32)
            nc.vector.tensor_mul(out=tl, in0=lb, in1=dd)
            u = sb.tile([H, W], F32)
            nc.vector.tensor_sub(out=u, in0=sp, in1=tl)

            # ---------------- spatial features (tile-major) ----------------
            # pre-scaled multipliers xs[c][e] = xb[c]/sqrt(e) (tensor_scalar: 4x mode)
            xs = {}
            for c in range(3):
                xs[(c, 1)] = xb[c]
                for e in range(2, DEG + 1):
                    t = sb.tile([H, W], BF16, tag=f"xs{c}_{e}")
                    nc.vector.tensor_scalar_mul(
                        out=t, in0=xb[c], scalar1=1.0 / math.sqrt(e)
                    )
                    xs[(c, e)] = t

            TT = sb.tile([H, NMON, W], BF16)
            nc.scalar.activation(out=TT[:, 0, :], in_=nr, func=AF.Exp)
            for (co, po, wid, ch, e) in MONOPS:
                mult = xs[(ch, e)]
                nc.vector.tensor_mul(
                    out=TT[:, co : co + wid, :],
                    in0=TT[:, po : po + wid, :],
                    in1=_bcast(mult[:], wid) if wid > 1 else mult[:],
                )

            mt1 = pm.tile([H, W], F32)
            for m in range(NMON):
                _mm = nc.tensor.matmul(
                    mt1,
                    lhsT=TT[:, m, :],
                    rhs=TT[:, m, :],
                    start=(m == 0),
                    stop=(m == NMON - 1),
                )
                # keep PE free for the bilateral pipe early on: slot the
                # spatial matmuls into PE gaps of the last bilateral groups
                anchor = NGRP - 4 + min(3, m * 4 // NMON)
                add_dep_helper(
                    _mm.ins, gmm_first[anchor], False, "interleave after bilateral"
                )

            # ---------------- M, q0, iterations ----------------
            # AT = I - M^T; q_{t+1} = q_t - M q_t; answer = 1^T q_5
            mt2s = sb.tile([H, W], F32)
            nc.vector.tensor_copy(out=mt2s, in_=mt2)
            MT = sb.tile([H, W], F32)
            nc.vector.tensor_add(out=MT, in0=mt1, in1=mt2s)

            q0p = ps.tile([H, 1], F32, tag="s")
            nc.tensor.matmul(q0p, lhsT=u, rhs=ones_col, start=True, stop=True)
            qcur = qpool.tile([H, 1], F32, tag="q")
            nc.vector.tensor_copy(out=qcur, in_=q0p)

            for it in range(5):
                yp = ps.tile([H, 1], F32, tag="s")
                nc.tensor.matmul(yp, lhsT=MT, rhs=qcur, start=True, stop=True)
                qn = qpool.tile([H, 1], F32, tag="q")
                nc.vector.tensor_sub(out=qn, in0=qcur, in1=yp)
                qcur = qn

            nc.sync.dma_start(out=out_d[:], in_=qcur)

    return nc


def _split_excess_waits(nc, max_waits=1, max_updates=1):
    """The walrus build in this container rejects instructions whose Events
    carry more than one semaphore wait (ISA Events has a single wait slot).
    Tile's sem assignment can attach several.  Split the extras onto
    same-engine NoOps placed immediately before (waits) / after (updates)
    the instruction; sequencers execute in order, so semantics are kept."""
    for fn in nc.m.functions:
        for bb in fn.blocks:
            ins = bb.instructions
            out = []
            changed = False
            for inst in ins:
                si = inst.sync_info
                if si is None:
                    out.append(inst)
                    continue
                waits = list(si.on_wait or [])
                updates = list(si.on_update or [])
                if len(waits) <= max_waits and len(updates) <= max_updates:
                    out.append(inst)
                    continue
                changed = True
                pre, post = [], []
                if len(waits) > max_waits:
                    for k, wt in enumerate(waits[:-max_waits]):
                        pre.append(
                            mybir.InstNoOp(
                                name=f"{inst.name}-w{k}",
                                engine=inst.engine,
                                bass_nofuse=True,
                                sync_info=mybir.SyncInfo(on_wait=[wt], on_update=[]),
                            )
                        )
                    waits = waits[-max_waits:]
                if len(updates) > max_updates:
                    for k, up in enumerate(updates[max_updates:]):
                        post.append(
                            mybir.InstNoOp(
                                name=f"{inst.name}-u{k}",
                                engine=inst.engine,
                                bass_nofuse=True,
                                sync_info=mybir.SyncInfo(on_wait=[], on_update=[up]),
                            )
                        )
                    updates = updates[:max_updates]
                inst.sync_info = mybir.SyncInfo(on_wait=waits, on_update=updates)
                out.extend(pre)
                out.append(inst)
                out.extend(post)
            if changed:
                bb.instructions = out
    return nc


_NC_CACHE = None


def kernel(logits, labels, images):
    global _NC_CACHE
    if _NC_CACHE is None:
        _NC_CACHE = _split_excess_waits(build_kernel())
    nc = _NC_CACHE

    import ml_dtypes

    logits = np.ascontiguousarray(np.asarray(logits, dtype=np.float32))
    labels_f = np.ascontiguousarray(np.asarray(labels).astype(np.float32))
    images = np.asarray(images, dtype=np.float32)
    imc = images - 0.5
    im_b = np.ascontiguousarray(imc.astype(ml_dtypes.bfloat16))
    imT_b = np.ascontiguousarray(np.swapaxes(imc, 2, 3).astype(ml_dtypes.bfloat16))

    in_maps = [
        {
            "logits": logits[b],
            "labels": labels_f[b],
            "imb": im_b[b],
            "imtb": imT_b[b],
        }
        for b in range(NB)
    ]
    res = run_bass_kernel_spmd(nc, in_maps, core_ids=list(range(NB)))
    tot = 0.0
    for b in range(NB):
        tot += float(res.results[b]["out"].astype(np.float64).sum())
    return np.float32(tot / (NB * H * W))



# revision 5
# speedup vs baseline: 2.7547x; 2.7547x over previous
"""Trainium2 Bass kernel for the CustomCRFLoss problem.

Strategy (pure data parallel, one sample per NeuronCore, 8 cores):

Per sample the reference reduces to  answer = 1^T (I - M)^5 q0  with
    q0[j]  = sum_i unary[i,j],        unary = softplus(d) - label*d
    M[j,w] = M1[j,w] + M2[j,w]
    M1[j,w] = sum_i k(x_ij, x_iw)     (row pairs, Gaussian kernel)
    M2[j,w] = sum_i k(x_ij, x_wj)     (within-column pairs)

Degree-2 Taylor feature map phi_m (10 monomials, m=0..9):
    k(a,b) ~ sum_m phi_m(a) phi_m(b),  phi_m(a) = a^alpha/sqrt(alpha!) e^{-r/2}
(|<a,b>| <= 0.75 for centered [-.5,.5]^3 pixels; final rel err ~2e-4, measured.)

With T[m][i,j] = phi_m(x_ij) (native layout only -- no transposes needed):
    M1 = sum_m T[m]^T T[m]                       10 bf16 matmuls in PSUM
    s_m[j] = sum_i T[m][i,j]                     10 N=1 matmuls (ones rhs)
    (M2 q)[j] = sum_m s_m[j] * (T[m]^T q)[j]     per-partition dot
so each mean-field iteration is 11 N=1 matmuls (g_m = T[m]^T q into PSUM
columns, plus y1 = M1^T q) and ONE fused DVE tensor_tensor_reduce:
    q' = q + sum_k Sext[:,k] * G[:,k],   Sext = [-s_0..-s_9, -1]
The final q5 is DMAd out; the host sums 128 floats per core.
"""

import math

import numpy as np

import concourse.bass as bass
import concourse.tile as tile
from concourse import mybir
from concourse.bass_utils import run_bass_kernel_spmd

H = W = 128
NB = 8  # batch / cores
NM = 10  # deg-2 monomials in 3 vars

F32 = mybir.dt.float32
BF16 = mybir.dt.bfloat16
AF = mybir.ActivationFunctionType
ALU = mybir.AluOpType
AX = mybir.AxisListType

LN2_HALF = 0.5 * math.log(2.0)
SQRT2 = math.sqrt(2.0)


def _bcast(ap, wid):
    """[P,128] AP -> [P,wid,128] with a step-0 middle dim."""
    return bass.AP(
        tensor=ap.tensor,
        offset=ap.offset,
        ap=[list(ap.ap[0]), [0, wid], list(ap.ap[1])],
    )


def build_kernel():
    nc = bass.Bass()
    im_d = nc.dram_tensor("imb", (H, 3, W), BF16, kind="ExternalInput")
    lg_d = nc.dram_tensor("lgb", (H, 3, W), BF16, kind="ExternalInput")
    out_d = nc.dram_tensor("out", (1, H), F32, kind="ExternalOutput")

    with tile.TileContext(nc) as tc:
        with (
            tc.tile_pool(name="sb", bufs=1) as sb,
            tc.tile_pool(name="qp", bufs=2) as qp,
            tc.tile_pool(name="pm", bufs=1, space="PSUM") as pm,
            tc.tile_pool(name="psg", bufs=2, space="PSUM") as psg,
            tc.tile_pool(name="pss", bufs=1, space="PSUM") as pss,
        ):
            # -------- input DMAs (images first: they gate the long pipe) ----
            X = sb.tile([H, 3, W], BF16)
            nc.sync.dma_start(out=X, in_=im_d[:])
            L = sb.tile([H, 3, W], BF16)
            nc.scalar.dma_start(out=L, in_=lg_d[:])

            # -------- constants (run during the DMA wait) -------------------
            ones_b = sb.tile([H, 1], BF16)
            nc.vector.memset(ones_b, 1.0)
            Sext = sb.tile([H, NM + 1], F32)
            nc.vector.memset(Sext[:, NM : NM + 1], -1.0)
            nln2h = sb.tile([H, 1], F32)
            nc.gpsimd.memset(nln2h, -LN2_HALF)

            # -------- r = |x|^2 pipeline (DVE) ------------------------------
            sq = sb.tile([H, 3, W], BF16)
            nc.vector.tensor_mul(out=sq, in0=X, in1=X)
            r12 = sb.tile([H, W], BF16)
            nc.vector.tensor_add(out=r12, in0=sq[:, 0, :], in1=sq[:, 1, :])
            rr = sb.tile([H, W], BF16)
            nc.vector.tensor_add(out=rr, in0=r12, in1=sq[:, 2, :])

            # -------- gating: E0h = exp(-r/2)/sqrt(2), E0 = exp(-r/2) -------
            E0h = sb.tile([H, W], BF16)
            nc.scalar.activation(out=E0h, in_=rr, func=AF.Exp, scale=-0.5,
                                 bias=nln2h[:])
            TT = sb.tile([H, NM, W], BF16)
            nc.vector.tensor_scalar_mul(out=TT[:, 0, :], in0=E0h, scalar1=SQRT2)

            # -------- deg-2 features (square block first: frees M1 start) ---
            # m: 0=E0, 1..3=x_c E0, 4..6=x_c^2 E0/sqrt2, 7=x0x1E0, 8=x0x2E0,
            # 9=x1x2E0
            nc.vector.tensor_mul(out=TT[:, 4:7, :], in0=sq, in1=_bcast(E0h[:], 3))
            nc.vector.tensor_mul(out=TT[:, 1:4, :], in0=X,
                                 in1=_bcast(TT[:, 0, :], 3))
            nc.vector.tensor_mul(out=TT[:, 7:9, :], in0=X[:, 1:3, :],
                                 in1=_bcast(TT[:, 1, :], 2))
            nc.vector.tensor_mul(out=TT[:, 9, :], in0=X[:, 2, :], in1=TT[:, 2, :])

            # -------- M1 = sum_m T[m]^T T[m] (PSUM accumulate) --------------
            morder = [4, 5, 6, 0, 1, 2, 3, 7, 8, 9]  # squares ready first
            m1p = pm.tile([H, W], F32)
            for k, m in enumerate(morder):
                nc.tensor.matmul(
                    m1p,
                    lhsT=TT[:, m, :],
                    rhs=TT[:, m, :],
                    start=(k == 0),
                    stop=(k == NM - 1),
                )
            M1sb = sb.tile([H, W], BF16)
            nc.scalar.activation(out=M1sb, in_=m1p, func=AF.Copy)

            # -------- column sums s_m (N=1 matmuls, nearly free) ------------
            sp_ = pss.tile([H, NM], F32)
            for m in range(NM):
                nc.tensor.matmul(
                    sp_[:, m : m + 1], lhsT=TT[:, m, :], rhs=ones_b,
                    start=True, stop=True,
                )
            nc.vector.tensor_scalar_mul(out=Sext[:, 0:NM], in0=sp_, scalar1=-1.0)

            # -------- unary -> q0 (Pool + ACT, off the critical path) -------
            d = sb.tile([H, W], BF16)
            nc.gpsimd.tensor_sub(out=d, in0=L[:, 1, :], in1=L[:, 0, :])
            ed = sb.tile([H, W], F32)
            nc.scalar.activation(out=ed, in_=d, func=AF.Exp)
            spl = sb.tile([H, W], F32)
            nc.scalar.activation(out=spl, in_=ed, func=AF.Ln, bias=1.0)
            lbd = sb.tile([H, W], BF16)
            nc.gpsimd.tensor_mul(out=lbd, in0=L[:, 2, :], in1=d)
            u_b = sb.tile([H, W], BF16)
            nc.gpsimd.tensor_sub(out=u_b, in0=spl, in1=lbd)
            q0p = pss.tile([H, 1], F32)
            nc.tensor.matmul(q0p, lhsT=u_b, rhs=ones_b, start=True, stop=True)
            qb = qp.tile([H, 1], BF16, tag="qb")
            nc.vector.tensor_copy(out=qb, in_=q0p)
            qf = qp.tile([H, 1], F32, tag="qf")
            nc.vector.tensor_copy(out=qf, in_=q0p)

            # -------- 5 mean-field iterations -------------------------------
            # acc = sum_k Sext[:,k]*G[:,k] = -(M1+M2)q per partition, then
            # qf' = qf + acc (with qb' = bf16 of it, via STT out/accum pair).
            for it in range(5):
                gp = psg.tile([H, NM + 1], F32, tag="g")
                for m in range(NM):
                    nc.tensor.matmul(
                        gp[:, m : m + 1], lhsT=TT[:, m, :], rhs=qb,
                        start=True, stop=True,
                    )
                nc.tensor.matmul(
                    gp[:, NM : NM + 1], lhsT=M1sb, rhs=qb, start=True, stop=True
                )
                scr = qp.tile([H, NM + 1], F32, tag="scr")
                acc = qp.tile([H, 1], F32, tag="acc")
                nc.vector.scalar_tensor_tensor(
                    out=scr, in0=gp, scalar=1.0, in1=Sext,
                    op0=ALU.mult, op1=ALU.mult, accum_out=acc,
                )
                qb2 = qp.tile([H, 1], BF16, tag="qb")
                qf2 = qp.tile([H, 1], F32, tag="qf")
                nc.vector.scalar_tensor_tensor(
                    out=qb2, in0=acc, scalar=1.0, in1=qf,
                    op0=ALU.mult, op1=ALU.add, accum_out=qf2,
                )
                qb, qf = qb2, qf2

            nc.sync.dma_start(out=out_d[:], in_=qf)

    return nc


def _split_excess_waits(nc, max_waits=1, max_updates=1):
    """The walrus build in this container rejects instructions whose Events
    carry more than one semaphore wait (ISA Events has a single wait slot).
    Tile's sem assignment can attach several.  Split the extras onto
    same-engine NoOps placed immediately before (waits) / after (updates)
    the instruction; sequencers execute in order, so semantics are kept."""
    for fn in nc.m.functions:
        for bb in fn.blocks:
            ins = bb.instructions
            out = []
            changed = False
            for inst in ins:
                si = inst.sync_info
                if si is None:
                    out.append(inst)
                    continue
                waits = list(si.on_wait or [])
                updates = list(si.on_update or [])
                if len(waits) <= max_waits and len(updates) <= max_updates:
                    out.append(inst)
                    continue
                changed = True
                pre, post = [], []
                if len(waits) > max_waits:
                    for k, wt in enumerate(waits[:-max_waits]):
                        pre.append(
                            mybir.InstNoOp(
                                name=f"{inst.name}-w{k}",
                                engine=inst.engine,
                                bass_nofuse=True,
                                sync_info=mybir.SyncInfo(on_wait=[wt], on_update=[]),
                            )
                        )
                    waits = waits[-max_waits:]
                if len(updates) > max_updates:
                    for k, up in enumerate(updates[max_updates:]):
                        post.append(
                            mybir.InstNoOp(
                                name=f"{inst.name}-u{k}",
                                engine=inst.engine,
                                bass_nofuse=True,
                                sync_info=mybir.SyncInfo(on_wait=[], on_update=[up]),
                            )
                        )
                    updates = updates[:max_updates]
                inst.sync_info = mybir.SyncInfo(on_wait=waits, on_update=updates)
                out.extend(pre)
                out.append(inst)
                out.extend(post)
            if changed:
                bb.instructions = out
    return nc


_NC_CACHE = None


def kernel(logits, labels, images):
    global _NC_CACHE
    if _NC_CACHE is None:
        _NC_CACHE = _split_excess_waits(build_kernel())
    nc = _NC_CACHE

    import ml_dtypes

    logits = np.asarray(logits, dtype=np.float32)
    labels_f = np.asarray(labels).astype(np.float32)
    images = np.asarray(images, dtype=np.float32)
    imc = (images - 0.5).astype(ml_dtypes.bfloat16)
    # [b, i, c, j] packing for both inputs
    im_b = np.ascontiguousarray(np.swapaxes(imc, 1, 2))
    lg_pack = np.stack([logits[:, 0], logits[:, 1], labels_f], axis=2)
    lg_b = np.ascontiguousarray(lg_pack.astype(ml_dtypes.bfloat16))

    in_maps = [{"imb": im_b[b], "lgb": lg_b[b]} for b in range(NB)]
    res = run_bass_kernel_spmd(nc, in_maps, core_ids=list(range(NB)))
    tot = 0.0
    for b in range(NB):
        tot += float(res.results[b]["out"].astype(np.float64).sum())
    return np.float32(tot / (NB * H * W))
